# revision 1
# baseline (speedup 1.0000x reference)
"""Single-head attention on 8 Trainium2 NeuronCores.

Problem: x[4,4096,1024] @ {Wq,Wk,Wv}[1024,64] -> scaled-dot-product
attention per batch -> out[4,4096,64].

Sharding: core c handles batch b=c//2, query half h=c%2 (2048 queries),
with K/V over the full 4096-row sequence of its batch. No collectives:
each core receives its batch's x pre-transposed (and pre-permuted so its
own query half sits in columns 0:2048, keeping the SPMD graph identical
across cores).

Dataflow per core (all matmul operands contraction-on-partitions):
  xT [1024,4096] bf16 (host-transposed)
  QT[128,2048] = [Wq|Wq]^T xT(own half)   (lo+hi copies for row-packing)
  VT/KT[128,4096] = [Wv|Wk]^T xT          (VT rows 0:64, KT rows 64:128)
  KT_lo = KT moved to partitions 0:64 via SBUF->SBUF DMA
  V'[s-tile] = PE-transpose(VT) with a ones column appended (65 cols)
  scoresT[s,t] = KT(s-tile)^T QT : two K=64 matmuls row-packed per s-pair
  PT = exp(scoresT/8)  (ScalarE, scale fused; scores bounded ~|8| so no
       running-max is needed for fp32 softmax)
  outT[65,t] += V'[s]^T PT[s]  accumulated over all 32 s-tiles in PSUM;
       row 64 is the softmax denominator (ones column).
Host divides rows 0:64 by row 64 and transposes back.
"""

import numpy as np

B, T, E, D = 4, 4096, 1024, 64
HALF = T // 2  # queries per core
NCORES = 8

_compiled = {}


def _patch_tile_drain():
    """This walrus build accepts only one sem-wait on the TileContext exit
    drain; spread the waits across preceding nofuse NOPs instead."""
    import concourse.tile as tile
    import concourse.mybir as mybir
    from concourse.tile import ScopedClock

    if getattr(tile.TileContext, "_drain_patch_installed", False):
        return

    def _drain_and_barrier(self, tick_clock, wait_clock):
        nops = [
            self.nc.sync.nop(nofuse=True, hint=f"drain_wait_{i}") for i in range(26)
        ]
        drain_inst = self.nc.sync.drain()
        wait_clock.add_sem_waits(
            drain_inst.ins, ScopedClock({None: tick_clock.global_clock})
        )
        si = drain_inst.ins.sync_info
        if si is not None and len(si.on_wait) > 1:
            waits = list(si.on_wait)
            assert len(waits) - 1 <= len(nops), f"{len(waits)} drain waits"
            si.on_wait = [waits[-1]]
            for w, nop in zip(waits[:-1], nops):
                nop.ins.sync_info = mybir.SyncInfo(on_wait=[w], on_update=[])

        self.nc.all_engine_barrier()
        assert self.sems is not None
        popped = self.nc._tile_sem_poison_stack.pop()
        assert popped is self._sem_poison
        self.nc.clear_and_free_semaphores(list(self.sems.allocated().values()))
        self.nc.all_engine_barrier()

    tile.TileContext._drain_and_barrier = _drain_and_barrier
    tile.TileContext._drain_patch_installed = True


def _patch_ldw_opt():
    """Enable walrus LDWEIGHTS double-buffering (pull-ahead): the repo
    hardcodes --enable-ldw-opt=false, which serializes a ~100ns weight
    load in front of every matmul."""
    import concourse.bass_utils as bu

    if getattr(bu, "_ldw_opt_patched", False):
        return
    orig = bu.run_command

    def run_command(cmd, *a, **kw):
        cmd = [
            c.replace("--enable-ldw-opt=false", "--enable-ldw-opt=true")
            if isinstance(c, str)
            else c
            for c in cmd
        ]
        return orig(cmd, *a, **kw)

    bu.run_command = run_command
    bu._ldw_opt_patched = True


def _strip_ldweights(nc):
    """Drop pre-split InstLdweights (InstMatmult still carries the weights
    operand, so walrus self-loads); keeps their sync as NoOps. Required for
    --enable-ldw-opt=true, which rejects standalone Ldweights."""
    import concourse.mybir as mybir

    for fn in nc.m.functions:
        for blk in fn.blocks:
            new_insts = []
            for inst in blk.instructions:
                if type(inst).__name__ == "InstLdweights":
                    si = inst.sync_info
                    if si is not None and (si.on_wait or si.on_update):
                        nop = mybir.InstNoOp(
                            name=f"{inst.name}-ldwnop",
                            ins=[],
                            outs=[],
                            bass_is_fusable=False,
                        )
                        nop.engine = inst.engine
                        nop.sync_info = si
                        new_insts.append(nop)
                    continue
                new_insts.append(inst)
            blk.instructions[:] = new_insts


def _split_multi_waits(nc):
    """This walrus build accepts only one sem-wait per instruction; hoist
    extra waits onto same-engine NoOps inserted just before the owner."""
    import concourse.mybir as mybir

    for fn in nc.m.functions:
        for blk in fn.blocks:
            new_insts = []
            for inst in blk.instructions:
                si = inst.sync_info
                if si is not None and len(si.on_wait) > 1:
                    waits = list(si.on_wait)
                    si.on_wait = [waits[-1]]
                    for j, w in enumerate(waits[:-1]):
                        nop = mybir.InstNoOp(
                            name=f"{inst.name}-waitsplit-{j}",
                            ins=[],
                            outs=[],
                            bass_is_fusable=False,
                        )
                        nop.engine = inst.engine
                        nop.sync_info = mybir.SyncInfo(on_wait=[w], on_update=[])
                        new_insts.append(nop)
                new_insts.append(inst)
            blk.instructions[:] = new_insts


def _build_nc():
    import concourse.bass as bass
    import concourse.mybir as mybir
    from concourse.tile import TileContext
    from concourse.masks import make_identity

    _patch_tile_drain()

    fp32 = mybir.dt.float32
    bf16 = mybir.dt.bfloat16
    fp8 = mybir.dt.float8e4
    Exp = mybir.ActivationFunctionType.Exp
    ADD = mybir.AluOpType.add
    DROW = mybir.MatmulPerfMode.DoubleRow
    LNS = float(np.log(32.0))

    nc = bass.Bass()

    xT_ext = nc.declare_dram_parameter("xT", [E, T], bf16, isOutput=False)
    wqq_ext = nc.declare_dram_parameter("w_qq", [E, 128], bf16, isOutput=False)
    wvk_ext = nc.declare_dram_parameter("w_vk", [E, 128], bf16, isOutput=False)
    bqq_ext = nc.declare_dram_parameter("b_qq", [128, 1], fp32, isOutput=False)
    bvk_ext = nc.declare_dram_parameter("b_vk", [128, 1], fp32, isOutput=False)
    out_ext = nc.declare_dram_parameter("outT", [D + 1, HALF], fp32, isOutput=True)

    EC = E // 128  # 8 contraction chunks
    TJ = T // 512  # 8 column chunks over full T
    NS = T // 128  # 32 s-tiles
    TH = HALF // 1024  # 2 query half-chunks per core

    with TileContext(nc) as tc:
        with (
            tc.tile_pool(name="w", bufs=1) as wpool,
            tc.tile_pool(name="xt", bufs=16) as xtpool,
            tc.tile_pool(name="big", bufs=1) as bigpool,
            tc.tile_pool(name="pt", bufs=4) as ptpool,
            tc.tile_pool(name="oc", bufs=2) as ocpool,
            tc.tile_pool(name="ps_proj", bufs=1, space="PSUM") as pspj,
            tc.tile_pool(name="ps_s", bufs=2, space="PSUM") as pss,
            tc.tile_pool(name="ps_o", bufs=1, space="PSUM") as pso,
        ):
            # --- weights / constants (one DMA per weight matrix) ---
            wqq_sb = wpool.tile([128, EC * 128], bf16, tag="wqq")
            wvk_sb = wpool.tile([128, EC * 128], bf16, tag="wvk")
            nc.sync.dma_start(
                out=wqq_sb[:], in_=wqq_ext[:].rearrange("(c p) m -> p c m", p=128)
            )
            nc.sync.dma_start(
                out=wvk_sb[:], in_=wvk_ext[:].rearrange("(c p) m -> p c m", p=128)
            )
            ident = wpool.tile([64, 64], bf16, tag="ident")
            make_identity(nc, ident[:])
            ball_sb = wpool.tile([128, 2], fp32, tag="ball")
            nc.sync.dma_start(out=ball_sb[:, 0:1], in_=bqq_ext[:])
            nc.sync.dma_start(out=ball_sb[:, 1:2], in_=bvk_ext[:])
            bqq_sb = ball_sb[:, 0:1]
            bvk_sb = ball_sb[:, 1:2]

            # --- TensorE warm-up: keep PE busy through the DMA-bound
            # startup so HAM unthrottles before the projections begin ---
            warm_sb = wpool.tile([128, 512], bf16, tag="warm")
            nc.vector.memset(warm_sb[:], 0.0)
            for wi in range(30):
                pswarm = pspj.tile([128, 512], fp32, tag="psp", name=f"pswarm{wi}")
                nc.tensor.matmul(
                    pswarm[:], lhsT=warm_sb[:, 0:128], rhs=warm_sb[:]
                )

            qq_sb = bigpool.tile([128, HALF], bf16, tag="qq")
            vk_sb = bigpool.tile([128, T], bf16, tag="vk")
            ktlo_sb = bigpool.tile([64, T], bf16, tag="ktlo")
            # V' tiles: [128, 65] per s-tile, ones in column 64
            vp_sb = bigpool.tile([128, NS * 65], bf16, tag="vp")
            nc.vector.memset(vp_sb[:], 1.0)

            # xT viewed as [p, e-chunk, t] for single-DMA column loads
            xT3 = xT_ext[:].rearrange("(c p) t -> p c t", p=128)

            ps_outs = {}

            def emit_phase_a(tj):
                cols = slice(tj * 512, (tj + 1) * 512)
                # one DMA brings all 8 e-chunks of this t-column block;
                # e-chunk c lands at xt[:, 512c:512c+512]
                xt = xtpool.tile([128, EC * 512], bf16, tag="xt")
                nc.sync.dma_start(out=xt[:], in_=xT3[:, :, cols])
                if tj < TJ // 2:  # own query half: Q projection
                    psq = pspj.tile([128, 512], fp32, tag="psp")
                    for e in range(EC):
                        nc.tensor.matmul(
                            psq[:],
                            lhsT=wqq_sb[:, e * 128 : (e + 1) * 128],
                            rhs=xt[:, e * 512 : (e + 1) * 512],
                            start=(e == 0),
                            stop=(e == EC - 1),
                        )
                    nc.vector.tensor_scalar(
                        qq_sb[:, cols], psq[:], bqq_sb[:], None, op0=ADD
                    )
                psv = pspj.tile([128, 512], fp32, tag="psp")
                for e in range(EC):
                    nc.tensor.matmul(
                        psv[:],
                        lhsT=wvk_sb[:, e * 128 : (e + 1) * 128],
                        rhs=xt[:, e * 512 : (e + 1) * 512],
                        start=(e == 0),
                        stop=(e == EC - 1),
                    )
                nc.vector.tensor_scalar(
                    vk_sb[:, cols], psv[:], bvk_sb[:], None, op0=ADD
                )
                # KT to partitions 0:64 (cross-partition => DMA)
                nc.sync.dma_start(out=ktlo_sb[:, cols], in_=vk_sb[64:128, cols])
                # V' build: PE-transpose VT 128-column blocks of this chunk
                for si in range(4 * tj, 4 * (tj + 1)):
                    pvt = pspj.tile([128, 64], bf16, tag="pvt", name=f"pvt{si}")
                    nc.tensor.transpose(
                        pvt[:], vk_sb[0:64, si * 128 : (si + 1) * 128], ident[:]
                    )
                    nc.vector.tensor_copy(
                        out=vp_sb[:, si * 65 : si * 65 + 64], in_=pvt[:]
                    )

            def emit_attn(th, k):
                if th not in ps_outs:
                    ps_outs[th] = pso.tile([D + 1, 1024], fp32, tag="pso", name=f"pso{th}")
                ps_out = ps_outs[th]
                sA, sB = 2 * k, 2 * k + 1
                psa = pss.tile([128, 1024], fp32, tag="pss")
                psb = pss.tile([128, 1024], fp32, tag="pss")
                for half in range(2):
                    mc = slice(half * 512, (half + 1) * 512)
                    qcols = slice(
                        th * 1024 + half * 512, th * 1024 + half * 512 + 512
                    )
                    nc.tensor.matmul(
                        psa[:, mc],
                        lhsT=ktlo_sb[:, sA * 128 : (sA + 1) * 128],
                        rhs=qq_sb[0:64, qcols],
                    )
                    nc.tensor.matmul(
                        psb[:, mc],
                        lhsT=vk_sb[64:128, sB * 128 : (sB + 1) * 128],
                        rhs=qq_sb[64:128, qcols],
                    )
                pta = ptpool.tile([128, 1024], bf16, tag="pt")
                ptb = ptpool.tile([128, 1024], bf16, tag="pt")
                nc.scalar.activation(pta[:], psa[:], Exp, scale=0.125)
                nc.scalar.activation(ptb[:], psb[:], Exp, scale=0.125)
                for si, pt in ((sA, pta), (sB, ptb)):
                    for half in range(2):
                        mc = slice(half * 512, (half + 1) * 512)
                        nc.tensor.matmul(
                            ps_out[:, mc],
                            lhsT=vp_sb[:, si * 65 : (si + 1) * 65],
                            rhs=pt[:, mc],
                            start=(k == 0 and si == sA),
                            stop=(k == NS // 2 - 1 and si == sB),
                        )

            def emit_flush(th):
                tcols = slice(th * 1024, (th + 1) * 1024)
                oc = ocpool.tile([D + 1, 1024], fp32, tag="oc")
                nc.vector.tensor_copy(out=oc[:], in_=ps_outs[th][:])
                nc.sync.dma_start(out=out_ext[:, tcols], in_=oc[:])

            # interleaved emission: phase-C chunks go out as soon as their
            # K/V s-tiles and Q columns exist, overlapping the exp stream
            # with the remaining projection work.
            for tj in range(TJ):
                emit_phase_a(tj)
                if tj == 1:
                    for k in range(4):
                        emit_attn(0, k)
                elif tj >= 2:
                    emit_attn(0, 2 * tj)
                    emit_attn(0, 2 * tj + 1)
            emit_flush(0)
            for k in range(NS // 2):
                emit_attn(1, k)
            emit_flush(1)

    nc.finalize()
    _split_multi_waits(nc)
    return nc


def _get_nc():
    if "nc" not in _compiled:
        _compiled["nc"] = _build_nc()
    return _compiled["nc"]


def _make_in_maps(x, Wq, bq, Wk, bk, Wv, bv):
    import ml_dtypes

    bf16 = ml_dtypes.bfloat16
    w_qq = np.concatenate([Wq, Wq], axis=1).astype(bf16)  # [E, 128]
    w_vk = np.concatenate([Wv, Wk], axis=1).astype(bf16)  # [E, 128]
    b_qq = np.concatenate([bq, bq]).reshape(128, 1).astype(np.float32)
    b_vk = np.concatenate([bv, bk]).reshape(128, 1).astype(np.float32)

    xT = np.transpose(x, (0, 2, 1))  # [B, E, T]
    in_maps = []
    for c in range(NCORES):
        b, h = divmod(c, 2)
        xb = xT[b]
        if h == 0:
            xp = np.ascontiguousarray(xb).astype(bf16)
        else:
            # permute so the core's own query half is in columns 0:HALF
            xp = np.concatenate([xb[:, HALF:], xb[:, :HALF]], axis=1).astype(bf16)
        in_maps.append(
            {"xT": xp, "w_qq": w_qq, "w_vk": w_vk, "b_qq": b_qq, "b_vk": b_vk}
        )
    return in_maps


def _assemble(results):
    out = np.empty((B, T, D), np.float32)
    for c in range(NCORES):
        b, h = divmod(c, 2)
        ot = results[c]["outT"]  # [65, HALF]
        out[b, h * HALF : (h + 1) * HALF, :] = (ot[:D] / ot[D : D + 1]).T
    return out


def kernel(x, Wq, bq, Wk, bk, Wv, bv):
    x = np.asarray(x, dtype=np.float32)
    Wq = np.asarray(Wq, dtype=np.float32)
    Wk = np.asarray(Wk, dtype=np.float32)
    Wv = np.asarray(Wv, dtype=np.float32)
    bq = np.asarray(bq, dtype=np.float32)
    bk = np.asarray(bk, dtype=np.float32)
    bv = np.asarray(bv, dtype=np.float32)

    from concourse.bass_utils import run_bass_kernel_spmd

    in_maps = _make_in_maps(x, Wq, bq, Wk, bk, Wv, bv)
    nc = _get_nc()
    res = run_bass_kernel_spmd(nc, in_maps, list(range(NCORES)))
    return _assemble(res.results)



# revision 9
# speedup vs baseline: 1.1227x; 1.1227x over previous
"""Single-head attention on 8 Trainium2 NeuronCores.

Problem: x[4,4096,1024] @ {Wq,Wk,Wv}[1024,64] -> scaled-dot-product
attention per batch -> out[4,4096,64].

Sharding: core c handles batch b=c//2, query half h=c%2 (2048 queries),
with K/V over the full 4096-row sequence of its batch. No collectives:
each core receives its batch's x pre-transposed (and pre-permuted so its
own query half sits in columns 0:2048, keeping the SPMD graph identical
across cores).

Dataflow per core (all matmul operands contraction-on-partitions):
  xT [1024,4096] bf16 (host-transposed), DMA'd in 512-col pieces
  QT[128,2048] = [Wq|Wq]^T xT(own half)   (lo+hi copies for row-packing)
  VT/KT[128,4096] = [Wv|Wk]^T xT          (VT rows 0:64, KT rows 64:128)
  KT_lo = KT moved to partitions 0:64 via SBUF->SBUF DMA
  V'[s-tile] = XBAR-DMA-transpose of VT with a ones column appended
  scoresT[s,t]: K=64 matmuls; even s-tile on PE rows 0:63 (ktlo), odd
       s-tile on rows 64:127 (vk_sb) -> the two run concurrently.
  PT = exp(scoresT/8)  (ScalarE, scale fused; scores bounded ~|8| so no
       running-max is needed for fp32 softmax)
  outT[65,t] += V'[s]^T PT[s]  accumulated over all 32 s-tiles in PSUM;
       row 64 is the softmax denominator (ones column).

The kernel is ScalarE-bound (64 exp tiles ~ 71us busy), so everything
is scheduled around keeping the exp stream dense: the attention loop is
software-pipelined (step k's score matmuls are emitted before step k-1's
exp+AV so the in-order PE queue never head-of-line blocks the exp
stream), and the projection matmuls are chopped into 4-matmul quads slid
into per-step PE slack, finishing each K/V chunk just before the first
attention step that consumes it. Host divides rows 0:64 by row 64 and
transposes back.
"""

import numpy as np

B, T, E, D = 4, 4096, 1024, 64
HALF = T // 2  # queries per core
NCORES = 8

USE_LDW_OPT = False  # walrus ldw-opt miscompiles (wrong results on HW)
USE_DMA_TRANSPOSE = False

_compiled = {}


def _patch_tile_drain():
    """This walrus build accepts only one sem-wait on the TileContext exit
    drain; spread the waits across preceding nofuse NOPs instead."""
    import concourse.tile as tile
    import concourse.mybir as mybir
    from concourse.tile import ScopedClock

    if getattr(tile.TileContext, "_drain_patch_installed", False):
        return

    def _drain_and_barrier(self, tick_clock, wait_clock):
        nops = [
            self.nc.sync.nop(nofuse=True, hint=f"drain_wait_{i}") for i in range(26)
        ]
        drain_inst = self.nc.sync.drain()
        wait_clock.add_sem_waits(
            drain_inst.ins, ScopedClock({None: tick_clock.global_clock})
        )
        si = drain_inst.ins.sync_info
        if si is not None and len(si.on_wait) > 1:
            waits = list(si.on_wait)
            assert len(waits) - 1 <= len(nops), f"{len(waits)} drain waits"
            si.on_wait = [waits[-1]]
            for w, nop in zip(waits[:-1], nops):
                nop.ins.sync_info = mybir.SyncInfo(on_wait=[w], on_update=[])

        self.nc.all_engine_barrier()
        assert self.sems is not None
        popped = self.nc._tile_sem_poison_stack.pop()
        assert popped is self._sem_poison
        self.nc.clear_and_free_semaphores(list(self.sems.allocated().values()))
        self.nc.all_engine_barrier()

    tile.TileContext._drain_and_barrier = _drain_and_barrier
    tile.TileContext._drain_patch_installed = True


def _patch_ldw_opt():
    """Enable walrus LDWEIGHTS double-buffering (pull-ahead): the repo
    hardcodes --enable-ldw-opt=false, which serializes a ~100ns weight
    load in front of every matmul."""
    import concourse.bass_utils as bu

    if getattr(bu, "_ldw_opt_patched", False):
        return
    orig = bu.run_command

    def run_command(cmd, *a, **kw):
        cmd = [
            c.replace("--enable-ldw-opt=false", "--enable-ldw-opt=true")
            if isinstance(c, str)
            else c
            for c in cmd
        ]
        return orig(cmd, *a, **kw)

    bu.run_command = run_command
    bu._ldw_opt_patched = True


def _strip_ldweights(nc):
    """Drop pre-split InstLdweights (InstMatmult still carries the weights
    operand, so walrus self-loads); keeps their sync as NoOps. Required for
    --enable-ldw-opt=true, which rejects standalone Ldweights."""
    import concourse.mybir as mybir

    for fn in nc.m.functions:
        for blk in fn.blocks:
            new_insts = []
            for inst in blk.instructions:
                if type(inst).__name__ == "InstLdweights":
                    si = inst.sync_info
                    if si is not None and (si.on_wait or si.on_update):
                        nop = mybir.InstNoOp(
                            name=f"{inst.name}-ldwnop",
                            ins=[],
                            outs=[],
                            bass_is_fusable=False,
                        )
                        nop.engine = inst.engine
                        nop.sync_info = si
                        new_insts.append(nop)
                    continue
                new_insts.append(inst)
            blk.instructions[:] = new_insts


def _split_multi_waits(nc):
    """This walrus build accepts only one sem-wait per instruction; hoist
    extra waits onto same-engine NoOps inserted just before the owner."""
    import concourse.mybir as mybir

    for fn in nc.m.functions:
        for blk in fn.blocks:
            new_insts = []
            for inst in blk.instructions:
                si = inst.sync_info
                if si is not None and len(si.on_wait) > 1:
                    waits = list(si.on_wait)
                    si.on_wait = [waits[-1]]
                    for j, w in enumerate(waits[:-1]):
                        nop = mybir.InstNoOp(
                            name=f"{inst.name}-waitsplit-{j}",
                            ins=[],
                            outs=[],
                            bass_is_fusable=False,
                        )
                        nop.engine = inst.engine
                        nop.sync_info = mybir.SyncInfo(on_wait=[w], on_update=[])
                        new_insts.append(nop)
                new_insts.append(inst)
            blk.instructions[:] = new_insts


def _build_nc():
    import concourse.bass as bass
    import concourse.mybir as mybir
    from concourse.tile import TileContext
    from concourse.masks import make_identity

    _patch_tile_drain()
    if USE_LDW_OPT:
        _patch_ldw_opt()

    fp32 = mybir.dt.float32
    bf16 = mybir.dt.bfloat16
    Exp = mybir.ActivationFunctionType.Exp
    ADD = mybir.AluOpType.add

    nc = bass.Bass()

    xT_ext = nc.declare_dram_parameter("xT", [E, T], bf16, isOutput=False)
    wqq_ext = nc.declare_dram_parameter("w_qq", [E, 128], bf16, isOutput=False)
    wvk_ext = nc.declare_dram_parameter("w_vk", [E, 128], bf16, isOutput=False)
    bqq_ext = nc.declare_dram_parameter("b_qq", [128, 1], fp32, isOutput=False)
    bvk_ext = nc.declare_dram_parameter("b_vk", [128, 1], fp32, isOutput=False)
    out_ext = nc.declare_dram_parameter("outT", [D + 1, HALF], fp32, isOutput=True)

    EC = E // 128  # 8 contraction chunks
    CW = 1024  # K/V chunk width (columns of xT)
    NCH = T // CW  # 4 chunks
    NS = T // 128  # 32 s-tiles
    NK = NS // 2  # 16 s-tile pairs per query block
    NJUNK = 32  # PE warm-up matmuls during the startup DMA wait

    with TileContext(nc) as tc:
        with (
            tc.tile_pool(name="w", bufs=1) as wpool,
            tc.tile_pool(name="xt", bufs=8) as xtpool,
            tc.tile_pool(name="big", bufs=1) as bigpool,
            tc.tile_pool(name="pt", bufs=4) as ptpool,
            tc.tile_pool(name="oc", bufs=2) as ocpool,
            tc.tile_pool(name="ps", bufs=2, space="PSUM") as pspool,
            tc.tile_pool(name="pj", bufs=2, space="PSUM") as pjpool,
            tc.tile_pool(name="po", bufs=1, space="PSUM") as popool,
        ):
            # --- input DMAs: all xT piece triggers go out first so the
            # in-order sync queue never delays a load behind later work ---
            xT3 = xT_ext[:].rearrange("(c p) t -> p c t", p=128)
            xts = {}

            def emit_xt(cj, h):
                xt = xtpool.tile(
                    [128, EC * 512], bf16, tag="xt", name=f"xt{cj}_{h}"
                )
                pc = slice(cj * CW + h * 512, cj * CW + h * 512 + 512)
                nc.sync.dma_start(out=xt[:], in_=xT3[:, :, pc])
                xts[(cj, h)] = xt

            emit_xt(0, 0)
            wqq_sb = wpool.tile([128, EC * 128], bf16, tag="wqq")
            wvk_sb = wpool.tile([128, EC * 128], bf16, tag="wvk")
            nc.sync.dma_start(
                out=wqq_sb[:], in_=wqq_ext[:].rearrange("(c p) m -> p c m", p=128)
            )
            nc.sync.dma_start(
                out=wvk_sb[:], in_=wvk_ext[:].rearrange("(c p) m -> p c m", p=128)
            )
            emit_xt(0, 1)
            ball_sb = wpool.tile([128, 2], fp32, tag="ball")
            nc.sync.dma_start(out=ball_sb[:, 0:1], in_=bqq_ext[:])
            nc.sync.dma_start(out=ball_sb[:, 1:2], in_=bvk_ext[:])
            bqq_sb = ball_sb[:, 0:1]
            bvk_sb = ball_sb[:, 1:2]
            for cj in range(1, NCH):
                emit_xt(cj, 0)
                emit_xt(cj, 1)

            # --- PE warm-up during the DMA wait (HAM unthrottle) + exp
            # table preload on ScalarE ---
            jw_sb = wpool.tile([128, 64], bf16, tag="jw")
            nc.vector.memset(jw_sb[:], 0.0)
            tl1 = wpool.tile([128, 1], fp32, tag="tl1")
            nc.scalar.activation(tl1[:], jw_sb[:, 0:1], Exp)
            psj = pjpool.tile([128, 64], fp32, tag="pj", name="psjunk")
            for _ in range(NJUNK):
                nc.tensor.matmul(psj[0:64, 0:64], lhsT=jw_sb[:], rhs=jw_sb[:])

            qq_sb = bigpool.tile([128, HALF], bf16, tag="qq")
            vk_sb = bigpool.tile([128, T], bf16, tag="vk")
            ktlo_sb = bigpool.tile([64, T], bf16, tag="ktlo")
            # V' tiles: [128, 65] per s-tile, ones in column 64
            vp_sb = bigpool.tile([128, NS * 65], bf16, tag="vp")
            nc.vector.memset(vp_sb[:], 1.0)
            if not USE_DMA_TRANSPOSE:
                ident = wpool.tile([64, 64], bf16, tag="ident")
                make_identity(nc, ident[:])

            # --- projection pieces: per (chunk, q|vk, 512-col half):
            # two 4-matmul quads + bias-add (+ ktlo copy and V' transposes
            # for vk halves), slid into the attention loop's PE slack ---
            pj_tiles = {}

            def emit_quad(cj, kind, h, quad):
                key = (cj, kind, h)
                if quad == 0:
                    pj_tiles[key] = pjpool.tile(
                        [128, 512], fp32, tag="pj", name=f"pj{cj}{kind}{h}"
                    )
                pj = pj_tiles[key]
                w_sb = wqq_sb if kind == "q" else wvk_sb
                xt = xts[(cj, h)]
                for e in range(4 * quad, 4 * quad + 4):
                    nc.tensor.matmul(
                        pj[:],
                        lhsT=w_sb[:, e * 128 : (e + 1) * 128],
                        rhs=xt[:, e * 512 : (e + 1) * 512],
                        start=(e == 0),
                        stop=(e == EC - 1),
                    )
                if quad == 1:
                    cols = slice(cj * CW + h * 512, cj * CW + h * 512 + 512)
                    if kind == "q":
                        nc.vector.tensor_scalar(
                            qq_sb[:, cols], pj[:], bqq_sb[:], None, op0=ADD
                        )
                    else:
                        nc.vector.tensor_scalar(
                            vk_sb[:, cols], pj[:], bvk_sb[:], None, op0=ADD
                        )
                        # KT to partitions 0:64 (cross-partition => DMA)
                        nc.sync.dma_start(
                            out=ktlo_sb[:, cols], in_=vk_sb[64:128, cols]
                        )
                        # V' build for the 4 s-tiles of this half
                        for si in range(8 * cj + 4 * h, 8 * cj + 4 * h + 4):
                            if USE_DMA_TRANSPOSE:
                                nc.sync.dma_start_transpose(
                                    out=vp_sb[:, si * 65 : si * 65 + 64],
                                    in_=vk_sb[0:64, si * 128 : (si + 1) * 128],
                                )
                            else:
                                pvt = pjpool.tile(
                                    [128, 64], bf16, tag="pj", name=f"pvt{si}"
                                )
                                nc.tensor.transpose(
                                    pvt[:],
                                    vk_sb[0:64, si * 128 : (si + 1) * 128],
                                    ident[:],
                                )
                                nc.vector.tensor_copy(
                                    out=vp_sb[:, si * 65 : si * 65 + 64], in_=pvt[:]
                                )

            ps_out = {}

            def emit_scores(th, k):
                """Score matmuls for s-tile pair (2k, 2k+1) x query block th.
                Even tile contracts on PE rows 0:63, odd on 64:127 -> the
                pairs run concurrently in the array."""
                sA, sB = 2 * k, 2 * k + 1
                psa = pspool.tile([128, 1024], fp32, tag="ps", name=f"psa{th}_{k}")
                psb = pspool.tile([128, 1024], fp32, tag="ps", name=f"psb{th}_{k}")
                for half in range(2):
                    mc = slice(half * 512, (half + 1) * 512)
                    qcols = slice(
                        th * 1024 + half * 512, th * 1024 + half * 512 + 512
                    )
                    nc.tensor.matmul(
                        psa[:, mc],
                        lhsT=ktlo_sb[:, sA * 128 : (sA + 1) * 128],
                        rhs=qq_sb[0:64, qcols],
                    )
                    nc.tensor.matmul(
                        psb[:, mc],
                        lhsT=vk_sb[64:128, sB * 128 : (sB + 1) * 128],
                        rhs=qq_sb[64:128, qcols],
                    )
                return psa, psb

            def emit_expav(th, k, psa, psb):
                """exp + AV accumulation for step (th, k)."""
                if th not in ps_out:
                    ps_out[th] = popool.tile(
                        [D + 1, 1024], fp32, tag="po", name=f"pso{th}"
                    )
                po = ps_out[th]
                sA, sB = 2 * k, 2 * k + 1
                pta = ptpool.tile([128, 1024], bf16, tag="pt")
                ptb = ptpool.tile([128, 1024], bf16, tag="pt")
                nc.scalar.activation(pta[:], psa[:], Exp, scale=0.125)
                nc.scalar.activation(ptb[:], psb[:], Exp, scale=0.125)
                for si, pt in ((sA, pta), (sB, ptb)):
                    for half in range(2):
                        mc = slice(half * 512, (half + 1) * 512)
                        nc.tensor.matmul(
                            po[:, mc],
                            lhsT=vp_sb[:, si * 65 : (si + 1) * 65],
                            rhs=pt[:, mc],
                            start=(k == 0 and si == sA),
                            stop=(k == NK - 1 and si == sB),
                        )

            def emit_flush(th):
                tcols = slice(th * 1024, (th + 1) * 1024)
                oc = ocpool.tile([D + 1, 1024], fp32, tag="oc")
                nc.vector.tensor_copy(out=oc[:], in_=ps_out[th][:])
                nc.sync.dma_start(out=out_ext[:, tcols], in_=oc[:])
                del ps_out[th]

            # --- chunk 0 fully in the fill phase (Q first: it gates the
            # first scores; per-half so scores can start on half data) ---
            for h in range(2):
                emit_quad(0, "q", h, 0)
                emit_quad(0, "q", h, 1)
                emit_quad(0, "vk", h, 0)
                emit_quad(0, "vk", h, 1)

            # remaining chunks: one quad per attention step, each chunk
            # completing just before the first step that needs its s-tiles
            # (chunk c s-tiles are first used at step k=4c), and chunk 1's
            # Q (only needed by th=1, step index 16) last.
            quad_sched = {}
            order = (
                [(1, "vk", h, q) for h in range(2) for q in range(2)]
                + [(2, "vk", h, q) for h in range(2) for q in range(2)]
                + [(3, "vk", h, q) for h in range(2) for q in range(2)]
                + [(1, "q", h, q) for h in range(2) for q in range(2)]
            )
            for i, item in enumerate(order):
                quad_sched.setdefault(i, []).append(item)

            # Software-pipelined attention: emit scores(k) before
            # exp+AV(k-1) so the PE queue stays ahead of the exp stream.
            steps = [(0, k) for k in range(NK)] + [(1, k) for k in range(NK)]
            pend = None
            for si, (th, k) in enumerate(steps):
                psa, psb = emit_scores(th, k)
                for item in quad_sched.get(si, ()):
                    emit_quad(*item)
                if pend is not None:
                    emit_expav(*pend)
                    if pend[0] == 0 and pend[1] == NK - 1:
                        emit_flush(0)
                pend = (th, k, psa, psb)
            emit_expav(*pend)
            emit_flush(1)

    nc.finalize()
    if USE_LDW_OPT:
        _strip_ldweights(nc)
    _split_multi_waits(nc)
    return nc


def _get_nc():
    if "nc" not in _compiled:
        _compiled["nc"] = _build_nc()
    return _compiled["nc"]


def _make_in_maps(x, Wq, bq, Wk, bk, Wv, bv):
    import ml_dtypes

    bf16 = ml_dtypes.bfloat16
    w_qq = np.concatenate([Wq, Wq], axis=1).astype(bf16)  # [E, 128]
    w_vk = np.concatenate([Wv, Wk], axis=1).astype(bf16)  # [E, 128]
    b_qq = np.concatenate([bq, bq]).reshape(128, 1).astype(np.float32)
    b_vk = np.concatenate([bv, bk]).reshape(128, 1).astype(np.float32)

    xT = np.transpose(x, (0, 2, 1))  # [B, E, T]
    in_maps = []
    for c in range(NCORES):
        b, h = divmod(c, 2)
        xb = xT[b]
        if h == 0:
            xp = np.ascontiguousarray(xb).astype(bf16)
        else:
            # permute so the core's own query half is in columns 0:HALF
            xp = np.concatenate([xb[:, HALF:], xb[:, :HALF]], axis=1).astype(bf16)
        in_maps.append(
            {"xT": xp, "w_qq": w_qq, "w_vk": w_vk, "b_qq": b_qq, "b_vk": b_vk}
        )
    return in_maps


def _assemble(results):
    out = np.empty((B, T, D), np.float32)
    for c in range(NCORES):
        b, h = divmod(c, 2)
        ot = results[c]["outT"]  # [65, HALF]
        out[b, h * HALF : (h + 1) * HALF, :] = (ot[:D] / ot[D : D + 1]).T
    return out


def kernel(x, Wq, bq, Wk, bk, Wv, bv):
    x = np.asarray(x, dtype=np.float32)
    Wq = np.asarray(Wq, dtype=np.float32)
    Wk = np.asarray(Wk, dtype=np.float32)
    Wv = np.asarray(Wv, dtype=np.float32)
    bq = np.asarray(bq, dtype=np.float32)
    bk = np.asarray(bk, dtype=np.float32)
    bv = np.asarray(bv, dtype=np.float32)

    from concourse.bass_utils import run_bass_kernel_spmd

    in_maps = _make_in_maps(x, Wq, bq, Wk, bk, Wv, bv)
    nc = _get_nc()
    res = run_bass_kernel_spmd(nc, in_maps, list(range(NCORES)))
    return _assemble(res.results)
